# revision 1
# baseline (speedup 1.0000x reference)
"""GRU-D Trainium2 Bass kernel.

Strategy (data-parallel over batch on 8 NeuronCores, per sharding hint):
  - Each core gets BL=512 batch rows; weights replicated.
  - State kept transposed: [j (hidden, partition within 4 chunks along free), b].
  - Per time step, gate pre-activations are computed on the PE:
      psum = U^T-chunks @ (gamma*h) chunks  +  rank-3 "extras" matmul
    where the extras matmul contracts [xi_t; mask_t; ones] against
    [w_x; w_m; bias] columns, folding the scalar-input terms and biases
    into the same PSUM accumulation group.
  - gamma_h = exp(-relu(Wgh*it + bgh)) = min(exp(-(Wgh*it + bgh)), 1):
    rank-2 matmul (negated weights) -> ACT exp -> min on gpsimd.
  - Sigmoids are computed as tanh: sigmoid(x) = (1+tanh(x/2))/2, with the
    1/2 input scales folded into the weights and the output affine folded
    into the state-update algebra (state is stored as 2*h).  This keeps all
    ACT work in the single "exp_and_others" table set (exp+tanh) -- no ACT
    table reloads in the hot loop.
  - Time loop is a hardware For_i loop; per-step scalar rows (xi_t, mask_t,
    interval_t) are staged from internal DRAM (T-major, written once by a
    PE-transpose preprocessing pass) via dynamic-offset DMAs, replicated to
    partition strips {0,32,64,96} so the small matmuls can be packed into
    concurrent PE row-groups via tile_position.
  - Staging rows + extras weights are always bf16 (validated: full-bf16
    operand rounding gives ~3e-5 abs error vs fp32 reference); the big
    U matmuls run at MM_MODE precision.

Self-contained: hardcodes shapes from the problem spec.
"""

import os
import numpy as np
from contextlib import ExitStack

import concourse.bass as bass
import concourse.bacc as bacc
import concourse.mybir as mybir
import concourse.tile as tile
from concourse.masks import make_identity
from concourse.bass_utils import run_bass_kernel_spmd

# ---- problem constants ----
B, T, H = 4096, 512, 512
GATE = H + 2
NCORES = 8
BL = B // NCORES      # 512 batch rows per core
S = 2                 # independent batch streams per core (pipelining)
W = BL // S           # 256 free-dim width per stream
G = 16                # time steps per staging half
PAD = 2 * G           # zero rows appended to T-major staging tensors
NC = 4                # H/128 partition chunks
P = 128

F32 = mybir.dt.float32
BF16 = mybir.dt.bfloat16
F32R = mybir.dt.float32r

# matmul mode for the U (hidden-state) matmuls: "f32", "f32r", or "bf16"
MM_MODE = os.environ.get("GRUD_MM_MODE", "bf16")
# ablation for timing bisection: "", "nodma", "nopool", "mmonly", "empty"
ABLATE = os.environ.get("GRUD_ABLATE", "")

AL = mybir.AluOpType
AF = mybir.ActivationFunctionType


def _sdt():
    """storage dtype for the U-matmul moving operands (state casts)"""
    return BF16 if MM_MODE == "bf16" else F32


def _mmv(ap):
    """view a U-matmul operand AP with the dtype the matmul should run at"""
    if MM_MODE == "f32r":
        return ap.bitcast(F32R)
    return ap


def build_module(t_steps=T, reps=1):
    assert t_steps % (2 * G) == 0
    sdt = _sdt()
    nc = bacc.Bacc(None, target_bir_lowering=False, debug=False)

    # ---- I/O ----
    x_d = nc.declare_dram_parameter("x", [BL, T], F32, isOutput=False)
    xl_d = nc.declare_dram_parameter("x_last", [BL, T], F32, isOutput=False)
    it_d = nc.declare_dram_parameter("interval", [BL, T], F32, isOutput=False)
    m_d = nc.declare_dram_parameter("mask", [BL, T], F32, isOutput=False)
    wgx_d = nc.declare_dram_parameter("Wgx", [1, 1], F32, isOutput=False)
    bgx_d = nc.declare_dram_parameter("bgx", [1], F32, isOutput=False)
    wgh_d = nc.declare_dram_parameter("Wgh", [H, 1], F32, isOutput=False)
    bgh_d = nc.declare_dram_parameter("bgh", [H], F32, isOutput=False)
    wz_d = nc.declare_dram_parameter("Wz", [H, GATE], F32, isOutput=False)
    bz_d = nc.declare_dram_parameter("bz", [H], F32, isOutput=False)
    wr_d = nc.declare_dram_parameter("Wr", [H, GATE], F32, isOutput=False)
    br_d = nc.declare_dram_parameter("br", [H], F32, isOutput=False)
    wh_d = nc.declare_dram_parameter("Wh", [H, GATE], F32, isOutput=False)
    bh_d = nc.declare_dram_parameter("bh", [H], F32, isOutput=False)
    wo_d = nc.declare_dram_parameter("Wo", [1, H], F32, isOutput=False)
    bo_d = nc.declare_dram_parameter("bo", [1], F32, isOutput=False)
    out_d = nc.declare_dram_parameter("out", [BL, 1], F32, isOutput=True)

    # internal T-major staging tensor (+pad so loop-tail prefetches stay in
    # bounds).  Components along dim1: 0=xi, 1=mask, 2=ones, 3=interval, 4=ones
    stgT_d = nc.dram_tensor("stgT", [T + PAD, 5, BL], BF16)
    # dram bounce for the extras/gamma weight tile (partition-scatter)
    exw_d = nc.dram_tensor("exw_dram", [P, H], BF16)

    gate_w = [wz_d, wr_d, wh_d]
    gate_b = [bz_d, br_d, bh_d]
    # scale folded into lhsT weights: z/r see tanh(u/2) (so 0.5), state carries
    # 2*h (so another 0.5 on the U part); extras see only the 0.5 tanh-halving.
    u_scale = [0.25, 0.25, 0.25]
    ex_scale = [0.5, 0.5, 1.0]

    with ExitStack() as ctx:
        tc = ctx.enter_context(tile.TileContext(nc))
        consts = ctx.enter_context(tc.tile_pool(name="consts", bufs=1))
        work = ctx.enter_context(tc.tile_pool(name="work", bufs=2))
        psum = ctx.enter_context(tc.tile_pool(name="psum", bufs=2, space="PSUM"))
        psum_b = ctx.enter_context(tc.tile_pool(name="psumb", bufs=2, space="PSUM"))
        psum_s = [psum, psum_b]

        ident = consts.tile([P, P], F32, tag="ident")
        make_identity(nc, ident[:])

        # ---------- fixed tiles ----------
        # extras/gamma stationary weights, strip layout on partitions:
        #  32g+0: w_x*s, 32g+1: w_m*s, 32g+2: b*s (g in {z,r,h}); 96: -Wgh, 97: -bgh
        exw = consts.tile([P, H], BF16, tag="exw")
        ut = [consts.tile([P, 16 * P], sdt, tag=f"ut{g}", name=f"ut{g}")
              for g in range(3)]
        wo_sb = consts.tile([P, NC], F32, tag="wo")
        bo_sb = consts.tile([1, 1], F32, tag="bo")
        wgx_bc = consts.tile([P, 1], F32, tag="wgx")
        bgx_bc = consts.tile([P, 1], F32, tag="bgx")
        scratch = consts.tile([P, H], F32, tag="scratch")
        # staging tiles [strip-partitions, G*W]; 2 halves x S streams
        stg = [[consts.tile([P, G * W], BF16, tag=f"stg{h}{s}",
                            name=f"stg{h}{s}") for s in range(S)]
               for h in range(2)]
        # ping-pong state (stored as 2*h_true), [j-chunk-major free]
        hst = [[consts.tile([P, NC * W], F32, tag=f"h{s}{p}", name=f"h{s}{p}")
                for p in range(2)]
               for s in range(S)]

        for s in range(S):
            nc.vector.memset(hst[s][0][:], 0.0)

        # ---------- preprocessing phase A: xi + T-major staging ----------
        with ExitStack() as pre:
            prep = pre.enter_context(tc.tile_pool(name="prep", bufs=1))
            # load inputs b-major: [p=b%128, (bchunk, t)]
            bm = {}
            for name, d in (("x", x_d), ("xl", xl_d), ("it", it_d), ("m", m_d)):
                tl = prep.tile([P, NC * T], F32, tag=f"bm_{name}",
                               name=f"bm_{name}")
                # one DMA for all 4 chunks: [(c p) t] -> [p (c t)]
                nc.sync.dma_start(
                    tl[:].rearrange("p (c t) -> p c t", c=NC),
                    d[:].rearrange("(c p) t -> p c t", c=NC))
                bm[name] = tl

            # scalar broadcasts
            nc.sync.dma_start(wgx_bc[:], wgx_d[0:1, 0:1].broadcast_to([P, 1]))
            nc.sync.dma_start(bgx_bc[:], bgx_d[:].unsqueeze(0).broadcast_to([P, 1]))

            # x_mean = sum(x*m)/sum(m) per row -> [128, NC]
            num = prep.tile([P, NC], F32, tag="num")
            den = prep.tile([P, NC], F32, tag="den")
            xm = prep.tile([P, NC], F32, tag="xm")
            prod = prep.tile([P, T], F32, tag="prod")
            for c in range(NC):
                cs = slice(c * T, (c + 1) * T)
                nc.vector.tensor_mul(prod[:], bm["x"][:, cs], bm["m"][:, cs])
                nc.vector.tensor_reduce(num[:, c:c + 1], prod[:],
                                        mybir.AxisListType.X, AL.add)
                nc.vector.tensor_reduce(den[:, c:c + 1], bm["m"][:, cs],
                                        mybir.AxisListType.X, AL.add)
            nc.vector.reciprocal(den[:], den[:])
            nc.vector.tensor_mul(xm[:], num[:], den[:])

            # gamma_x = exp(-relu(wgx*it + bgx))
            # u = xm + gx*(xl - xm);  xi = u + m*(x - u)
            ta = prep.tile([P, NC * T], F32, tag="ta")   # holds xl-xm, then u
            tb = prep.tile([P, NC * T], F32, tag="tb")   # holds gx, then xi
            nc.scalar.activation(tb[:], bm["it"][:], AF.Relu,
                                 bias=bgx_bc[:], scale=wgx_bc[:])
            nc.scalar.activation(tb[:], tb[:], AF.Exp, scale=-1.0)
            for c in range(NC):
                cs = slice(c * T, (c + 1) * T)
                nc.vector.tensor_scalar(ta[:, cs], bm["xl"][:, cs],
                                        xm[:, c:c + 1], None, AL.subtract)
            nc.vector.tensor_mul(ta[:], tb[:], ta[:])
            for c in range(NC):
                cs = slice(c * T, (c + 1) * T)
                nc.vector.tensor_scalar(ta[:, cs], ta[:, cs],
                                        xm[:, c:c + 1], None, AL.add)
            # now ta = u; build xi in tb (gx dead)
            nc.vector.tensor_sub(tb[:], bm["x"][:], ta[:])
            nc.vector.tensor_mul(tb[:], bm["m"][:], tb[:])
            nc.vector.tensor_add(tb[:], tb[:], ta[:])

            # transpose xi/m/it to T-major dram components (bf16)
            stage = prep.tile([P, BL], BF16, tag="stage")
            for src, comp in ((tb, 0), (bm["m"], 1), (bm["it"], 3)):
                for tcb in range(T // P):
                    for bc in range(NC):
                        pst = psum.tile([P, NC * W], F32, tag="ps")
                        nc.tensor.matmul(pst[:, 0:P],
                                         src[:, bc * T + tcb * P:
                                             bc * T + (tcb + 1) * P],
                                         ident[:], is_transpose=True)
                        nc.vector.tensor_copy(stage[:, bc * P:(bc + 1) * P],
                                              pst[:, 0:P])
                    nc.sync.dma_start(
                        stgT_d[tcb * P:(tcb + 1) * P, comp:comp + 1, :],
                        stage[:].unsqueeze(1))
                # zero pad rows
                zz = prep.tile([P, BL], BF16, tag="stage")
                nc.vector.memset(zz[:], 0.0)
                nc.sync.dma_start(stgT_d[T:T + PAD, comp:comp + 1, :],
                                  zz[0:PAD, :].unsqueeze(1))
            # ones components (2 and 4), including pad rows
            ones_t = prep.tile([P, BL], BF16, tag="stage")
            nc.vector.memset(ones_t[:], 1.0)
            for comp in (2, 4):
                for r0 in range(0, T + PAD, P):
                    rn = min(P, T + PAD - r0)
                    nc.sync.dma_start(stgT_d[r0:r0 + rn, comp:comp + 1, :],
                                      ones_t[0:rn, :].unsqueeze(1))

        # ---------- preprocessing phase B: gate weights ----------
        with ExitStack() as pre:
            prep = pre.enter_context(tc.tile_pool(name="prepw", bufs=1))
            wsb = prep.tile([P, NC * GATE], F32, tag="wsb")
            colt = prep.tile([P, H], BF16, tag="colt")
            rowb = prep.tile([1, H], BF16, tag="rowb")

            def row_to_exw(dram_src_row, scale, dst_row):
                """dram row -> scratch[0:1] -> scale/cast -> exw_d[dst_row]"""
                nc.sync.dma_start(scratch[0:1, :], dram_src_row)
                nc.vector.tensor_scalar(rowb[0:1, :], scratch[0:1, :],
                                        scale, None, AL.mult)
                nc.sync.dma_start(exw_d[dst_row:dst_row + 1, :], rowb[0:1, :])

            for g in range(3):
                for jc in range(NC):
                    nc.sync.dma_start(wsb[:, jc * GATE:(jc + 1) * GATE],
                                      gate_w[g][jc * P:(jc + 1) * P, :])
                # U^T tiles: lhsT[(kc,jc)] = (Wg[j, 1+k]).T * u_scale
                for jc in range(NC):
                    for kc in range(NC):
                        pst = psum.tile([P, NC * W], F32, tag="ps")
                        nc.tensor.matmul(
                            pst[:, 0:P],
                            wsb[:, jc * GATE + 1 + kc * P:
                                jc * GATE + 1 + (kc + 1) * P],
                            ident[:], is_transpose=True)
                        nc.vector.tensor_scalar(
                            ut[g][:, (kc * NC + jc) * P:(kc * NC + jc + 1) * P],
                            pst[:, 0:P], u_scale[g], None, AL.mult)
                # extras rows: columns 0 and GATE-1 of Wg, via strided transpose
                for jc in range(NC):
                    pst = psum.tile([P, NC * W], F32, tag="ps")
                    incol = wsb[:, jc * GATE: (jc + 1) * GATE: GATE - 1]
                    nc.tensor.matmul(pst[0:2, 0:P], incol, ident[:],
                                     is_transpose=True)
                    nc.vector.tensor_scalar(colt[0:2, jc * P:(jc + 1) * P],
                                            pst[0:2, 0:P], ex_scale[g],
                                            None, AL.mult)
                nc.sync.dma_start(exw_d[32 * g:32 * g + 2, :], colt[0:2, :])
                row_to_exw(gate_b[g][:].unsqueeze(0), ex_scale[g], 32 * g + 2)
            # gamma rows (negated)
            row_to_exw(wgh_d[:, 0:1].transpose([1, 0]), -1.0, 96)
            row_to_exw(bgh_d[:].unsqueeze(0), -1.0, 97)
            # gather the strip tile from dram (only the written row groups)
            for g in range(3):
                nc.sync.dma_start(exw[32 * g:32 * g + 3, :],
                                  exw_d[32 * g:32 * g + 3, :])
            nc.sync.dma_start(exw[96:98, :], exw_d[96:98, :])
            # output head: Wo^T/4 column chunks, bo/2
            for kc in range(NC):
                nc.sync.dma_start(wo_sb[:, kc:kc + 1],
                                  wo_d[0:1, kc * P:(kc + 1) * P].transpose([1, 0]))
            nc.vector.tensor_scalar(wo_sb[:], wo_sb[:], 0.25, None, AL.mult)
            nc.sync.dma_start(bo_sb[:], bo_d[:].unsqueeze(0))
            nc.vector.tensor_scalar(bo_sb[:], bo_sb[:], 0.5, None, AL.mult)

        # ---------- staging DMA helpers ----------
        def fill_stg(h, s, rows_src, eng=None):
            """rows_src(c0, c1): [G, c1-c0, W] source block (comps c0:c1)"""
            eng = eng or nc.sync
            t0 = stg[h][s]
            for strip in (0, 32, 64):
                eng.dma_start(t0[strip:strip + 3, :],
                              rows_src(0, 3).transpose([1, 0, 2]))
            eng.dma_start(t0[96:98, :], rows_src(3, 5).transpose([1, 0, 2]))

        # prologue: fill both halves for t in [0, 2G)
        def prologue():
            for h in range(2):
                for s in range(S):
                    fill_stg(h, s, lambda c0, c1, h=h, s=s:
                             stgT_d[h * G:(h + 1) * G, c0:c1,
                                    s * W:(s + 1) * W])
        prologue()

        # ---------- per-step emission ----------
        def step_part1(s, t_loc, stgt, u):
            p = t_loc % 2
            h_in = hst[s][p]
            bw = u * W

            # gamma: rank-2 matmuls into psum strips
            if ABLATE != "mmonly_nosmalls":
                psg = psum_s[s].tile([P, NC * W], F32, tag="ps")
                for jc in range(NC):
                    nc.tensor.matmul(psg[:, jc * W:(jc + 1) * W],
                                     exw[96:98, jc * P:(jc + 1) * P],
                                     stgt[96:98, bw:bw + W],
                                     start=True, stop=True,
                                     tile_position=(96, 0))
            if ABLATE.startswith("mmonly"):
                hgm = hst[s][0].bitcast(BF16)[:, 0:NC * W]
                res = {"hg": None, "hg_mm": hgm}
                for name, g in (("r", 1), ("z", 0)):
                    ps = psum_s[s].tile([P, NC * W], F32, tag="ps")
                    for jc in range(NC):
                        if ABLATE != "mmonly_smalls":
                            for kc in range(NC):
                                nc.tensor.matmul(
                                    ps[:, jc * W:(jc + 1) * W],
                                    _mmv(ut[g][:, (kc * NC + jc) * P:
                                               (kc * NC + jc + 1) * P]),
                                    _mmv(hgm[:, kc * W:(kc + 1) * W]),
                                    start=(kc == 0), stop=False)
                        if ABLATE != "mmonly_nosmalls":
                            nc.tensor.matmul(
                                ps[:, jc * W:(jc + 1) * W],
                                exw[32 * g:32 * g + 3, jc * P:(jc + 1) * P],
                                stgt[32 * g:32 * g + 3, bw:bw + W],
                                start=(ABLATE == "mmonly_smalls"), stop=True,
                                tile_position=(32 * g, 0))
                    res["ps" + name] = ps
                res["thz"] = None
                res["rh2"] = hgm
                return res
            e = work.tile([P, NC * W], F32, tag="e")
            nc.scalar.activation(e[:], psg[:], AF.Exp)
            if ABLATE == "nopool":
                nc.vector.tensor_scalar(e[:], e[:], 1.0, None, AL.min)
            else:
                nc.gpsimd.tensor_scalar(e[:], e[:], 1.0, None, AL.min)

            hgm = None
            if MM_MODE == "bf16":
                hgm = work.tile([P, NC * W], BF16, tag="hgm")
                nc.vector.tensor_mul(hgm[:], e[:], h_in[:])
            hg = work.tile([P, NC * W], F32, tag="hg")
            if ABLATE == "nopool":
                nc.vector.tensor_mul(hg[:], e[:], h_in[:])
            else:
                nc.gpsimd.tensor_mul(hg[:], e[:], h_in[:])
            hg_mm = hgm if MM_MODE == "bf16" else hg

            res = {"hg": hg, "hg_mm": hg_mm}
            # r then z matmul groups (r first: it gates the h~ chain)
            for name, g in (("r", 1), ("z", 0)):
                ps = psum_s[s].tile([P, NC * W], F32, tag="ps")
                for jc in range(NC):
                    for kc in range(NC):
                        nc.tensor.matmul(
                            ps[:, jc * W:(jc + 1) * W],
                            _mmv(ut[g][:, (kc * NC + jc) * P:
                                       (kc * NC + jc + 1) * P]),
                            _mmv(hg_mm[:, kc * W:(kc + 1) * W]),
                            start=(kc == 0), stop=False)
                    nc.tensor.matmul(
                        ps[:, jc * W:(jc + 1) * W],
                        exw[32 * g:32 * g + 3, jc * P:(jc + 1) * P],
                        stgt[32 * g:32 * g + 3, bw:bw + W],
                        start=False, stop=True, tile_position=(32 * g, 0))
                res["ps" + name] = ps
            thr = work.tile([P, NC * W], sdt, tag="thr")
            nc.scalar.activation(thr[:], res["psr"][:], AF.Tanh)
            thz = work.tile([P, NC * W], F32, tag="thz")
            nc.scalar.activation(thz[:], res["psz"][:], AF.Tanh)
            rh2 = work.tile([P, NC * W], sdt, tag="rh2")
            # (thr + 1) * hg_mm  == 2*r*hg_stored
            nc.vector.scalar_tensor_tensor(rh2[:], thr[:], 1.0, hg_mm[:],
                                           AL.add, AL.mult)
            res["thz"] = thz
            res["rh2"] = rh2
            return res

        def step_part2(s, t_loc, stgt, u, r1):
            p = t_loc % 2
            h_out = hst[s][1 - p]
            bw = u * W
            psh = psum_s[s].tile([P, NC * W], F32, tag="ps")
            for jc in range(NC):
                if ABLATE != "mmonly_smalls":
                    for kc in range(NC):
                        nc.tensor.matmul(
                            psh[:, jc * W:(jc + 1) * W],
                            _mmv(ut[2][:, (kc * NC + jc) * P:
                                       (kc * NC + jc + 1) * P]),
                            _mmv(r1["rh2"][:, kc * W:(kc + 1) * W]),
                            start=(kc == 0), stop=False)
                if ABLATE != "mmonly_nosmalls":
                    nc.tensor.matmul(
                        psh[:, jc * W:(jc + 1) * W],
                        exw[64:67, jc * P:(jc + 1) * P],
                        stgt[64:67, bw:bw + W],
                        start=(ABLATE == "mmonly_smalls"), stop=True,
                        tile_position=(64, 0))
            if ABLATE.startswith("mmonly"):
                return
            ht = work.tile([P, NC * W], F32, tag="ht")
            nc.scalar.activation(ht[:], psh[:], AF.Tanh)
            # A = (thz+1)*ht ; Bm = (thz-1)*hg ; h' = A - 0.5*Bm
            at = work.tile([P, NC * W], F32, tag="at")
            nc.vector.scalar_tensor_tensor(at[:], r1["thz"][:], 1.0, ht[:],
                                           AL.add, AL.mult)
            bm_ = work.tile([P, NC * W], F32, tag="bm")
            nc.vector.scalar_tensor_tensor(bm_[:], r1["thz"][:], 1.0,
                                           r1["hg"][:], AL.subtract, AL.mult)
            nc.vector.scalar_tensor_tensor(h_out[:], bm_[:], -0.5, at[:],
                                           AL.mult, AL.add)

        # ---------- hardware time loop ----------
        for _rep in range(reps):
          if _rep:
              prologue()
          with tc.For_i(0, t_steps, 2 * G) as iv:
              for h in range(2):
                  for u in range(G):
                      t_loc = h * G + u
                      if ABLATE == "empty":
                          continue
                      for s in range(S):
                          r1 = step_part1(s, t_loc, stg[h][s], u)
                          step_part2(s, t_loc, stg[h][s], u, r1)
                  # refill this half's staging for iteration iv+2G
                  for s in range(S):
                      eng = [[nc.sync, nc.sync], [nc.gpsimd, nc.scalar]][h][s]
                      fill_stg(h, s, lambda c0, c1, h=h, s=s:
                               stgT_d[2 * G + h * G:, c0:c1,
                                      s * W:(s + 1) * W][bass.ds(iv, G)],
                               eng=eng)

        # ---------- output head ----------
        for s in range(S):
            h_fin = hst[s][0]
            pso = psum_s[s].tile([P, NC * W], F32, tag="ps")
            for kc in range(NC):
                nc.tensor.matmul(pso[0:1, 0:W], wo_sb[:, kc:kc + 1],
                                 h_fin[:, kc * W:(kc + 1) * W],
                                 start=(kc == 0), stop=(kc == NC - 1))
            tho = work.tile([1, W], F32, tag="tho")
            nc.scalar.activation(tho[:], pso[0:1, 0:W], AF.Tanh,
                                 bias=bo_sb[0:1, 0:1])
            oo = work.tile([1, W], F32, tag="oo")
            nc.vector.tensor_scalar(oo[:], tho[:], 0.5, 0.5, AL.mult, AL.add)
            nc.sync.dma_start(out_d[s * W:(s + 1) * W, :].transpose([1, 0]),
                              oo[0:1, :])

    nc.finalize()
    return nc


_cached = {}


def _get_module():
    key = MM_MODE
    if key not in _cached:
        _cached[key] = build_module()
    return _cached[key]


def kernel(**inputs):
    nc = _get_module()
    core_ids = list(range(NCORES))
    in_maps = []
    for c in range(NCORES):
        sl = slice(c * BL, (c + 1) * BL)
        m = {
            "x": np.ascontiguousarray(inputs["x"][sl], np.float32),
            "x_last": np.ascontiguousarray(inputs["x_last"][sl], np.float32),
            "interval": np.ascontiguousarray(inputs["interval"][sl], np.float32),
            "mask": np.ascontiguousarray(inputs["mask"][sl], np.float32),
        }
        for wname in ("Wgx", "bgx", "Wgh", "bgh", "Wz", "bz", "Wr", "br",
                      "Wh", "bh", "Wo", "bo"):
            m[wname] = np.ascontiguousarray(inputs[wname], np.float32)
        in_maps.append(m)
    res = run_bass_kernel_spmd(nc, in_maps, core_ids)
    outs = [res.results[c]["out"].reshape(BL, 1) for c in range(NCORES)]
    return np.concatenate(outs, axis=0).astype(np.float32)



# revision 10
# speedup vs baseline: 7.7765x; 7.7765x over previous
"""GRU-D Trainium2 Bass kernel (v2: minimized host->device wire traffic).

Strategy (data-parallel over batch on 8 NeuronCores, per sharding hint):
  - Each core gets BL=512 batch rows; weights replicated.
  - Wire-traffic optimization (the axon tunnel runs at ~43 MB/s with ~80ms
    per-transfer latency, so host->device bytes dominated the old wall time):
      * x and x_last are combined host-side into xc = where(mask, x, x_last)
        (the reference only reads x where mask==1 and x_last where mask==0),
        cast to bf16.
      * mask is packed into the mantissa LSB of bf16 interval (costs 1 ulp
        of interval precision; decoded exactly on device via bitwise ops).
      * data payload per call: ONE [B, 2T] bf16 array = 8.4 MB (vs 33.5 MB
        fp32 + 25 MB replicated fp32 weights before).
      * weights are cast to bf16 (they were consumed as bf16 by the matmuls
        anyway), shipped once, and cached on device keyed by content hash.
      * the jitted shard_map executable is built once per process and
        reused (the generic runner re-traced + re-lowered on every call).
  - State kept transposed: [j (hidden, partition within 4 chunks along free), b].
  - Per time step, gate pre-activations are computed on the PE:
      psum = U^T-chunks @ (gamma*h) chunks  +  rank-3 "extras" matmul
    where the extras matmul contracts [xi_t; mask_t; ones] against
    [w_x; w_m; bias] columns, folding the scalar-input terms and biases
    into the same PSUM accumulation group.
  - gamma_h = exp(-relu(Wgh*it + bgh)) = min(exp(-(Wgh*it + bgh)), 1):
    rank-2 matmul (negated weights) -> ACT exp -> min on gpsimd.
  - Sigmoids are computed as tanh: sigmoid(x) = (1+tanh(x/2))/2, with the
    1/2 input scales folded into the weights and the output affine folded
    into the state-update algebra (state is stored as 2*h).  This keeps all
    ACT work in the single "exp_and_others" table set (exp+tanh) -- no ACT
    table reloads in the hot loop.
  - Time loop is a hardware For_i loop; per-step scalar rows (xi_t, mask_t,
    interval_t) are staged from internal DRAM (T-major, written once by a
    PE-transpose preprocessing pass) via dynamic-offset DMAs, replicated to
    partition strips {0,32,64,96} so the small matmuls can be packed into
    concurrent PE row-groups via tile_position.

Self-contained: hardcodes shapes from the problem spec.
"""

import os
import hashlib
import numpy as np
import ml_dtypes
from contextlib import ExitStack

import jax
from jax.sharding import Mesh, PartitionSpec, NamedSharding
from jax.experimental.shard_map import shard_map

import concourse.bass as bass
import concourse.bacc as bacc
import concourse.mybir as mybir
import concourse.tile as tile
from concourse.masks import make_identity
from concourse.bass2jax import (_bass_exec_p, install_neuronx_cc_hook,
                                partition_id_tensor)

# ---- problem constants ----
B, T, H = 4096, 512, 512
GATE = H + 2
NCORES = 8
BL = B // NCORES      # 512 batch rows per core
S = 2                 # independent batch streams per core (pipelining)
W = BL // S           # 256 free-dim width per stream
G = 16                # time steps per staging half
PAD = 2 * G           # zero rows appended to T-major staging tensors
NC = 4                # H/128 partition chunks
P = 128

F32 = mybir.dt.float32
BF16 = mybir.dt.bfloat16
F32R = mybir.dt.float32r
U16 = mybir.dt.uint16
NPBF = ml_dtypes.bfloat16

# matmul mode for the U (hidden-state) matmuls: "f32", "f32r", or "bf16"
MM_MODE = os.environ.get("GRUD_MM_MODE", "bf16")
# ablation for timing bisection: "", "nodma", "nopool", "mmonly", "empty"
ABLATE = os.environ.get("GRUD_ABLATE", "")

AL = mybir.AluOpType
AF = mybir.ActivationFunctionType

# weight parameter order (must match declaration order in build_module)
W_NAMES = ("Wgx", "bgx", "Wgh", "bgh", "Wz", "bz", "Wr", "br",
           "Wh", "bh", "Wo", "bo")
# dtypes the device expects for each weight parameter
W_NPDT = {"Wgx": np.float32, "bgx": np.float32, "bo": np.float32}


def _wnp(name):
    return W_NPDT.get(name, NPBF)


def _sdt():
    """storage dtype for the U-matmul moving operands (state casts)"""
    return BF16 if MM_MODE == "bf16" else F32


def _mmv(ap):
    """view a U-matmul operand AP with the dtype the matmul should run at"""
    if MM_MODE == "f32r":
        return ap.bitcast(F32R)
    return ap


def build_module(t_steps=T, reps=1):
    assert t_steps % (2 * G) == 0
    sdt = _sdt()
    nc = bacc.Bacc(None, target_bir_lowering=False, debug=False)

    # ---- I/O ----
    # data: [:, 0:T] = xc (bf16), [:, T:2T] = interval with mask in LSB
    data_d = nc.declare_dram_parameter("data", [BL, 2 * T], BF16,
                                       isOutput=False)
    wgx_d = nc.declare_dram_parameter("Wgx", [1, 1], F32, isOutput=False)
    bgx_d = nc.declare_dram_parameter("bgx", [1], F32, isOutput=False)
    wgh_d = nc.declare_dram_parameter("Wgh", [H, 1], BF16, isOutput=False)
    bgh_d = nc.declare_dram_parameter("bgh", [H], BF16, isOutput=False)
    wz_d = nc.declare_dram_parameter("Wz", [H, GATE], BF16, isOutput=False)
    bz_d = nc.declare_dram_parameter("bz", [H], BF16, isOutput=False)
    wr_d = nc.declare_dram_parameter("Wr", [H, GATE], BF16, isOutput=False)
    br_d = nc.declare_dram_parameter("br", [H], BF16, isOutput=False)
    wh_d = nc.declare_dram_parameter("Wh", [H, GATE], BF16, isOutput=False)
    bh_d = nc.declare_dram_parameter("bh", [H], BF16, isOutput=False)
    wo_d = nc.declare_dram_parameter("Wo", [1, H], BF16, isOutput=False)
    bo_d = nc.declare_dram_parameter("bo", [1], F32, isOutput=False)
    out_d = nc.declare_dram_parameter("out", [BL, 1], F32, isOutput=True)

    # internal T-major staging tensor (+pad so loop-tail prefetches stay in
    # bounds).  Components along dim1: 0=xi, 1=mask, 2=ones, 3=interval, 4=ones
    stgT_d = nc.dram_tensor("stgT", [T + PAD, 5, BL], BF16)
    # dram bounce for the extras/gamma weight tile (partition-scatter)
    exw_d = nc.dram_tensor("exw_dram", [P, H], BF16)

    gate_w = [wz_d, wr_d, wh_d]
    gate_b = [bz_d, br_d, bh_d]
    # scale folded into lhsT weights: z/r see tanh(u/2) (so 0.5), state carries
    # 2*h (so another 0.5 on the U part); extras see only the 0.5 tanh-halving.
    u_scale = [0.25, 0.25, 0.25]
    ex_scale = [0.5, 0.5, 1.0]

    with ExitStack() as ctx:
        tc = ctx.enter_context(tile.TileContext(nc))
        consts = ctx.enter_context(tc.tile_pool(name="consts", bufs=1))
        work = ctx.enter_context(tc.tile_pool(name="work", bufs=2))
        psum = ctx.enter_context(tc.tile_pool(name="psum", bufs=2, space="PSUM"))
        psum_b = ctx.enter_context(tc.tile_pool(name="psumb", bufs=2, space="PSUM"))
        psum_s = [psum, psum_b]

        ident = consts.tile([P, P], F32, tag="ident")
        make_identity(nc, ident[:])

        # ---------- fixed tiles ----------
        # extras/gamma stationary weights, strip layout on partitions:
        #  32g+0: w_x*s, 32g+1: w_m*s, 32g+2: b*s (g in {z,r,h}); 96: -Wgh, 97: -bgh
        exw = consts.tile([P, H], BF16, tag="exw")
        ut = [consts.tile([P, 16 * P], sdt, tag=f"ut{g}", name=f"ut{g}")
              for g in range(3)]
        wo_sb = consts.tile([P, NC], F32, tag="wo")
        bo_sb = consts.tile([1, 1], F32, tag="bo")
        wgx_bc = consts.tile([P, 1], F32, tag="wgx")
        bgx_bc = consts.tile([P, 1], F32, tag="bgx")
        # staging tiles [strip-partitions, G*W]; 2 halves x S streams
        stg = [[consts.tile([P, G * W], BF16, tag=f"stg{h}{s}",
                            name=f"stg{h}{s}") for s in range(S)]
               for h in range(2)]
        # ping-pong state (stored as 2*h_true), [j-chunk-major free]
        hst = [[consts.tile([P, NC * W], F32, tag=f"h{s}{p}", name=f"h{s}{p}")
                for p in range(2)]
               for s in range(S)]

        for s in range(S):
            nc.vector.memset(hst[s][0][:], 0.0)

        # ---------- preprocessing phase A: decode + xi + T-major staging ----
        with ExitStack() as pre:
            prep = pre.enter_context(tc.tile_pool(name="prep", bufs=1))
            # load packed inputs b-major: [p=b%128, (bchunk, t)]
            bmx = prep.tile([P, NC * T], BF16, tag="bmx", name="bmx")
            nc.sync.dma_start(
                bmx[:].rearrange("p (c t) -> p c t", c=NC),
                data_d[:, 0:T].rearrange("(c p) t -> p c t", c=NC))
            bmi = prep.tile([P, NC * T], BF16, tag="bmi", name="bmi")
            nc.sync.dma_start(
                bmi[:].rearrange("p (c t) -> p c t", c=NC),
                data_d[:, T:2 * T].rearrange("(c p) t -> p c t", c=NC))

            # scalar broadcasts
            nc.sync.dma_start(wgx_bc[:], wgx_d[0:1, 0:1].broadcast_to([P, 1]))
            nc.sync.dma_start(bgx_bc[:], bgx_d[:].unsqueeze(0).broadcast_to([P, 1]))

            # decode: xc (f32), it (f32, LSB noise ok), m = LSB of itm (exact)
            xc = prep.tile([P, NC * T], F32, tag="xc", name="xc")
            nc.vector.tensor_copy(xc[:], bmx[:])
            itf = prep.tile([P, NC * T], F32, tag="itf", name="itf")
            nc.vector.tensor_copy(itf[:], bmi[:])
            mu = prep.tile([P, NC * T], BF16, tag="mu", name="mu")
            nc.vector.tensor_scalar(mu.bitcast(U16)[:], bmi.bitcast(U16)[:],
                                    1, None, AL.bitwise_and)
            mf = prep.tile([P, NC * T], F32, tag="mf", name="mf")
            nc.vector.tensor_scalar(mf[:], mu.bitcast(U16)[:], 1, None,
                                    AL.is_equal)

            # x_mean = sum(xc*m)/sum(m) per row -> [128, NC]
            num = prep.tile([P, NC], F32, tag="num")
            den = prep.tile([P, NC], F32, tag="den")
            xm = prep.tile([P, NC], F32, tag="xm")
            prod = prep.tile([P, T], F32, tag="prod")
            for c in range(NC):
                cs = slice(c * T, (c + 1) * T)
                nc.vector.tensor_mul(prod[:], xc[:, cs], mf[:, cs])
                nc.vector.tensor_reduce(num[:, c:c + 1], prod[:],
                                        mybir.AxisListType.X, AL.add)
                nc.vector.tensor_reduce(den[:, c:c + 1], mf[:, cs],
                                        mybir.AxisListType.X, AL.add)
            nc.vector.reciprocal(den[:], den[:])
            nc.vector.tensor_mul(xm[:], num[:], den[:])

            # gamma_x = exp(-relu(wgx*it + bgx))
            # u = xm + gx*(xc - xm);  xi = u + m*(xc - u)
            ta = prep.tile([P, NC * T], F32, tag="ta")   # holds xc-xm, then u
            tb = prep.tile([P, NC * T], F32, tag="tb")   # holds gx, then xi
            nc.scalar.activation(tb[:], itf[:], AF.Relu,
                                 bias=bgx_bc[:], scale=wgx_bc[:])
            nc.scalar.activation(tb[:], tb[:], AF.Exp, scale=-1.0)
            for c in range(NC):
                cs = slice(c * T, (c + 1) * T)
                nc.vector.tensor_scalar(ta[:, cs], xc[:, cs],
                                        xm[:, c:c + 1], None, AL.subtract)
            nc.vector.tensor_mul(ta[:], tb[:], ta[:])
            for c in range(NC):
                cs = slice(c * T, (c + 1) * T)
                nc.vector.tensor_scalar(ta[:, cs], ta[:, cs],
                                        xm[:, c:c + 1], None, AL.add)
            # now ta = u; build xi in tb (gx dead)
            nc.vector.tensor_sub(tb[:], xc[:], ta[:])
            nc.vector.tensor_mul(tb[:], mf[:], tb[:])
            nc.vector.tensor_add(tb[:], tb[:], ta[:])

            # transpose xi/m/it to T-major dram components (bf16)
            stage = prep.tile([P, BL], BF16, tag="stage")
            for src, comp in ((tb, 0), (mf, 1), (itf, 3)):
                for tcb in range(T // P):
                    for bc in range(NC):
                        pst = psum.tile([P, NC * W], F32, tag="ps")
                        nc.tensor.matmul(pst[:, 0:P],
                                         src[:, bc * T + tcb * P:
                                             bc * T + (tcb + 1) * P],
                                         ident[:], is_transpose=True)
                        nc.vector.tensor_copy(stage[:, bc * P:(bc + 1) * P],
                                              pst[:, 0:P])
                    nc.sync.dma_start(
                        stgT_d[tcb * P:(tcb + 1) * P, comp:comp + 1, :],
                        stage[:].unsqueeze(1))
                # zero pad rows
                zz = prep.tile([P, BL], BF16, tag="stage")
                nc.vector.memset(zz[:], 0.0)
                nc.sync.dma_start(stgT_d[T:T + PAD, comp:comp + 1, :],
                                  zz[0:PAD, :].unsqueeze(1))
            # ones components (2 and 4), including pad rows
            ones_t = prep.tile([P, BL], BF16, tag="stage")
            nc.vector.memset(ones_t[:], 1.0)
            for comp in (2, 4):
                for r0 in range(0, T + PAD, P):
                    rn = min(P, T + PAD - r0)
                    nc.sync.dma_start(stgT_d[r0:r0 + rn, comp:comp + 1, :],
                                      ones_t[0:rn, :].unsqueeze(1))

        # ---------- preprocessing phase B: gate weights ----------
        with ExitStack() as pre:
            prep = pre.enter_context(tc.tile_pool(name="prepw", bufs=1))
            wsb_h = prep.tile([P, NC * GATE], BF16, tag="wsbh")
            wsb = prep.tile([P, NC * GATE], F32, tag="wsb")
            colt = prep.tile([P, H], BF16, tag="colt")
            rowb = prep.tile([1, H], BF16, tag="rowb")
            scratch = prep.tile([1, H], BF16, tag="scratch")

            def row_to_exw(dram_src_row, scale, dst_row):
                """dram row -> scratch[0:1] -> scale/cast -> exw_d[dst_row]"""
                nc.sync.dma_start(scratch[0:1, :], dram_src_row)
                nc.vector.tensor_scalar(rowb[0:1, :], scratch[0:1, :],
                                        scale, None, AL.mult)
                nc.sync.dma_start(exw_d[dst_row:dst_row + 1, :], rowb[0:1, :])

            for g in range(3):
                for jc in range(NC):
                    nc.sync.dma_start(wsb_h[:, jc * GATE:(jc + 1) * GATE],
                                      gate_w[g][jc * P:(jc + 1) * P, :])
                nc.vector.tensor_copy(wsb[:], wsb_h[:])
                # U^T tiles: lhsT[(kc,jc)] = (Wg[j, 1+k]).T * u_scale
                for jc in range(NC):
                    for kc in range(NC):
                        pst = psum.tile([P, NC * W], F32, tag="ps")
                        nc.tensor.matmul(
                            pst[:, 0:P],
                            wsb[:, jc * GATE + 1 + kc * P:
                                jc * GATE + 1 + (kc + 1) * P],
                            ident[:], is_transpose=True)
                        nc.vector.tensor_scalar(
                            ut[g][:, (kc * NC + jc) * P:(kc * NC + jc + 1) * P],
                            pst[:, 0:P], u_scale[g], None, AL.mult)
                # extras rows: columns 0 and GATE-1 of Wg, via strided transpose
                for jc in range(NC):
                    pst = psum.tile([P, NC * W], F32, tag="ps")
                    incol = wsb[:, jc * GATE: (jc + 1) * GATE: GATE - 1]
                    nc.tensor.matmul(pst[0:2, 0:P], incol, ident[:],
                                     is_transpose=True)
                    nc.vector.tensor_scalar(colt[0:2, jc * P:(jc + 1) * P],
                                            pst[0:2, 0:P], ex_scale[g],
                                            None, AL.mult)
                nc.sync.dma_start(exw_d[32 * g:32 * g + 2, :], colt[0:2, :])
                row_to_exw(gate_b[g][:].unsqueeze(0), ex_scale[g], 32 * g + 2)
            # gamma rows (negated)
            row_to_exw(wgh_d[:, 0:1].transpose([1, 0]), -1.0, 96)
            row_to_exw(bgh_d[:].unsqueeze(0), -1.0, 97)
            # gather the strip tile from dram (only the written row groups)
            for g in range(3):
                nc.sync.dma_start(exw[32 * g:32 * g + 3, :],
                                  exw_d[32 * g:32 * g + 3, :])
            nc.sync.dma_start(exw[96:98, :], exw_d[96:98, :])
            # output head: Wo^T/4 column chunks, bo/2
            wo_sbb = prep.tile([P, NC], BF16, tag="wob")
            for kc in range(NC):
                nc.sync.dma_start(wo_sbb[:, kc:kc + 1],
                                  wo_d[0:1, kc * P:(kc + 1) * P].transpose([1, 0]))
            nc.vector.tensor_scalar(wo_sb[:], wo_sbb[:], 0.25, None, AL.mult)
            nc.sync.dma_start(bo_sb[:], bo_d[:].unsqueeze(0))
            nc.vector.tensor_scalar(bo_sb[:], bo_sb[:], 0.5, None, AL.mult)

        # ---------- staging DMA helpers ----------
        def fill_stg(h, s, rows_src, eng=None):
            """rows_src(c0, c1): [G, c1-c0, W] source block (comps c0:c1)"""
            eng = eng or nc.sync
            t0 = stg[h][s]
            for strip in (0, 32, 64):
                eng.dma_start(t0[strip:strip + 3, :],
                              rows_src(0, 3).transpose([1, 0, 2]))
            eng.dma_start(t0[96:98, :], rows_src(3, 5).transpose([1, 0, 2]))

        # prologue: fill both halves for t in [0, 2G)
        def prologue():
            for h in range(2):
                for s in range(S):
                    fill_stg(h, s, lambda c0, c1, h=h, s=s:
                             stgT_d[h * G:(h + 1) * G, c0:c1,
                                    s * W:(s + 1) * W])
        prologue()

        # ---------- per-step emission ----------
        def step_part1(s, t_loc, stgt, u):
            p = t_loc % 2
            h_in = hst[s][p]
            bw = u * W

            # gamma: rank-2 matmuls into psum strips
            if ABLATE != "mmonly_nosmalls":
                psg = psum_s[s].tile([P, NC * W], F32, tag="ps")
                for jc in range(NC):
                    nc.tensor.matmul(psg[:, jc * W:(jc + 1) * W],
                                     exw[96:98, jc * P:(jc + 1) * P],
                                     stgt[96:98, bw:bw + W],
                                     start=True, stop=True,
                                     tile_position=(96, 0))
            if ABLATE.startswith("mmonly"):
                hgm = hst[s][0].bitcast(BF16)[:, 0:NC * W]
                res = {"hg": None, "hg_mm": hgm}
                for name, g in (("r", 1), ("z", 0)):
                    ps = psum_s[s].tile([P, NC * W], F32, tag="ps")
                    for jc in range(NC):
                        if ABLATE != "mmonly_smalls":
                            for kc in range(NC):
                                nc.tensor.matmul(
                                    ps[:, jc * W:(jc + 1) * W],
                                    _mmv(ut[g][:, (kc * NC + jc) * P:
                                               (kc * NC + jc + 1) * P]),
                                    _mmv(hgm[:, kc * W:(kc + 1) * W]),
                                    start=(kc == 0), stop=False)
                        if ABLATE != "mmonly_nosmalls":
                            nc.tensor.matmul(
                                ps[:, jc * W:(jc + 1) * W],
                                exw[32 * g:32 * g + 3, jc * P:(jc + 1) * P],
                                stgt[32 * g:32 * g + 3, bw:bw + W],
                                start=(ABLATE == "mmonly_smalls"), stop=True,
                                tile_position=(32 * g, 0))
                    res["ps" + name] = ps
                res["thz"] = None
                res["rh2"] = hgm
                return res
            e = work.tile([P, NC * W], F32, tag="e")
            nc.scalar.activation(e[:], psg[:], AF.Exp)
            if ABLATE == "nopool":
                nc.vector.tensor_scalar(e[:], e[:], 1.0, None, AL.min)
            else:
                nc.gpsimd.tensor_scalar(e[:], e[:], 1.0, None, AL.min)

            hgm = None
            if MM_MODE == "bf16":
                hgm = work.tile([P, NC * W], BF16, tag="hgm")
                nc.vector.tensor_mul(hgm[:], e[:], h_in[:])
            hg = work.tile([P, NC * W], F32, tag="hg")
            if ABLATE == "nopool":
                nc.vector.tensor_mul(hg[:], e[:], h_in[:])
            else:
                nc.gpsimd.tensor_mul(hg[:], e[:], h_in[:])
            hg_mm = hgm if MM_MODE == "bf16" else hg

            res = {"hg": hg, "hg_mm": hg_mm}
            # r then z matmul groups (r first: it gates the h~ chain)
            for name, g in (("r", 1), ("z", 0)):
                ps = psum_s[s].tile([P, NC * W], F32, tag="ps")
                for jc in range(NC):
                    for kc in range(NC):
                        nc.tensor.matmul(
                            ps[:, jc * W:(jc + 1) * W],
                            _mmv(ut[g][:, (kc * NC + jc) * P:
                                       (kc * NC + jc + 1) * P]),
                            _mmv(hg_mm[:, kc * W:(kc + 1) * W]),
                            start=(kc == 0), stop=False)
                    nc.tensor.matmul(
                        ps[:, jc * W:(jc + 1) * W],
                        exw[32 * g:32 * g + 3, jc * P:(jc + 1) * P],
                        stgt[32 * g:32 * g + 3, bw:bw + W],
                        start=False, stop=True, tile_position=(32 * g, 0))
                res["ps" + name] = ps
            thr = work.tile([P, NC * W], sdt, tag="thr")
            nc.scalar.activation(thr[:], res["psr"][:], AF.Tanh)
            thz = work.tile([P, NC * W], F32, tag="thz")
            nc.scalar.activation(thz[:], res["psz"][:], AF.Tanh)
            rh2 = work.tile([P, NC * W], sdt, tag="rh2")
            # (thr + 1) * hg_mm  == 2*r*hg_stored
            nc.vector.scalar_tensor_tensor(rh2[:], thr[:], 1.0, hg_mm[:],
                                           AL.add, AL.mult)
            res["thz"] = thz
            res["rh2"] = rh2
            return res

        def step_part2(s, t_loc, stgt, u, r1):
            p = t_loc % 2
            h_out = hst[s][1 - p]
            bw = u * W
            psh = psum_s[s].tile([P, NC * W], F32, tag="ps")
            for jc in range(NC):
                if ABLATE != "mmonly_smalls":
                    for kc in range(NC):
                        nc.tensor.matmul(
                            psh[:, jc * W:(jc + 1) * W],
                            _mmv(ut[2][:, (kc * NC + jc) * P:
                                       (kc * NC + jc + 1) * P]),
                            _mmv(r1["rh2"][:, kc * W:(kc + 1) * W]),
                            start=(kc == 0), stop=False)
                if ABLATE != "mmonly_nosmalls":
                    nc.tensor.matmul(
                        psh[:, jc * W:(jc + 1) * W],
                        exw[64:67, jc * P:(jc + 1) * P],
                        stgt[64:67, bw:bw + W],
                        start=(ABLATE == "mmonly_smalls"), stop=True,
                        tile_position=(64, 0))
            if ABLATE.startswith("mmonly"):
                return
            ht = work.tile([P, NC * W], F32, tag="ht")
            nc.scalar.activation(ht[:], psh[:], AF.Tanh)
            # A = (thz+1)*ht ; Bm = (thz-1)*hg ; h' = A - 0.5*Bm
            at = work.tile([P, NC * W], F32, tag="at")
            nc.vector.scalar_tensor_tensor(at[:], r1["thz"][:], 1.0, ht[:],
                                           AL.add, AL.mult)
            bm_ = work.tile([P, NC * W], F32, tag="bm")
            nc.vector.scalar_tensor_tensor(bm_[:], r1["thz"][:], 1.0,
                                           r1["hg"][:], AL.subtract, AL.mult)
            nc.vector.scalar_tensor_tensor(h_out[:], bm_[:], -0.5, at[:],
                                           AL.mult, AL.add)

        # ---------- hardware time loop ----------
        for _rep in range(reps):
          if _rep:
              prologue()
          with tc.For_i(0, t_steps, 2 * G) as iv:
              for h in range(2):
                  for u in range(G):
                      t_loc = h * G + u
                      if ABLATE == "empty":
                          continue
                      for s in range(S):
                          r1 = step_part1(s, t_loc, stg[h][s], u)
                          step_part2(s, t_loc, stg[h][s], u, r1)
                  # refill this half's staging for iteration iv+2G
                  for s in range(S):
                      eng = [[nc.sync, nc.sync], [nc.gpsimd, nc.scalar]][h][s]
                      fill_stg(h, s, lambda c0, c1, h=h, s=s:
                               stgT_d[2 * G + h * G:, c0:c1,
                                      s * W:(s + 1) * W][bass.ds(iv, G)],
                               eng=eng)

        # ---------- output head ----------
        for s in range(S):
            h_fin = hst[s][0]
            pso = psum_s[s].tile([P, NC * W], F32, tag="ps")
            for kc in range(NC):
                nc.tensor.matmul(pso[0:1, 0:W], wo_sb[:, kc:kc + 1],
                                 h_fin[:, kc * W:(kc + 1) * W],
                                 start=(kc == 0), stop=(kc == NC - 1))
            tho = work.tile([1, W], F32, tag="tho")
            nc.scalar.activation(tho[:], pso[0:1, 0:W], AF.Tanh,
                                 bias=bo_sb[0:1, 0:1])
            oo = work.tile([1, W], F32, tag="oo")
            nc.vector.tensor_scalar(oo[:], tho[:], 0.5, 0.5, AL.mult, AL.add)
            nc.sync.dma_start(out_d[s * W:(s + 1) * W, :].transpose([1, 0]),
                              oo[0:1, :])

    nc.finalize()
    return nc


# ---------------- host-side runner (cached jit + device weight cache) -------

class _Runner:
    def __init__(self, t_steps=T, reps=1):
        install_neuronx_cc_hook()
        self.nc = build_module(t_steps, reps)
        nc = self.nc
        partition_name = (nc.partition_id_tensor.name
                          if nc.partition_id_tensor else None)
        in_names, out_names, out_avals, zero_outs = [], [], [], []
        for alloc in nc.m.functions[0].allocations:
            if not isinstance(alloc, mybir.MemoryLocationSet):
                continue
            name = alloc.memorylocations[0].name
            if alloc.kind == "ExternalInput":
                if name != partition_name:
                    in_names.append(name)
            elif alloc.kind == "ExternalOutput":
                shape = tuple(alloc.tensor_shape)
                dtype = mybir.dt.np(alloc.dtype)
                out_names.append(name)
                out_avals.append(jax.core.ShapedArray(shape, dtype))
                zero_outs.append(np.zeros(shape, dtype))
        self.in_names = in_names
        self.out_names = out_names
        self.n_params = len(in_names)
        self.zero_outs = zero_outs
        in_names_all = in_names + out_names
        if partition_name is not None:
            in_names_all = in_names_all + [partition_name]
        out_avals_t = tuple(out_avals)

        devices = jax.devices()[:NCORES]
        assert len(devices) == NCORES
        self.mesh = Mesh(np.asarray(devices), ("core",))
        self.repl = NamedSharding(self.mesh, PartitionSpec())

        def _body(*args):
            operands = list(args)
            if partition_name is not None:
                operands.append(partition_id_tensor())
            outs = _bass_exec_p.bind(
                *operands, out_avals=out_avals_t, in_names=tuple(in_names_all),
                out_names=tuple(out_names),
                lowering_input_output_aliases=(),
                sim_require_finite=True, sim_require_nnan=True, nc=nc)
            return tuple(outs)

        def spec(name):
            # data + donated output buffers are batch-sharded; weights
            # replicated (full copy visible on every core)
            return (PartitionSpec("core") if name == "data"
                    else PartitionSpec())

        in_specs = tuple(spec(n) for n in in_names) + \
            (PartitionSpec("core"),) * len(out_names)
        out_specs = (PartitionSpec("core"),) * len(out_names)
        donate = tuple(range(self.n_params, self.n_params + len(out_names)))
        self.sharded = jax.jit(
            shard_map(_body, mesh=self.mesh, in_specs=in_specs,
                      out_specs=out_specs, check_rep=False),
            donate_argnums=donate, keep_unused=True)
        self._wkey = None
        self._wdev = None

    def weights_on_device(self, inputs):
        hsh = hashlib.blake2b(digest_size=16)
        for name in W_NAMES:
            a = np.asarray(inputs[name])
            hsh.update(a.tobytes())
        key = hsh.digest()
        if key != self._wkey:
            wdev = {}
            for name in W_NAMES:
                arr = np.ascontiguousarray(
                    np.asarray(inputs[name], np.float32)).astype(_wnp(name))
                wdev[name] = jax.device_put(arr, self.repl)
            jax.block_until_ready(list(wdev.values()))
            self._wdev = wdev
            self._wkey = key
        return self._wdev

    def pack_data(self, inputs):
        x = np.asarray(inputs["x"], np.float32)
        xl = np.asarray(inputs["x_last"], np.float32)
        it = np.asarray(inputs["interval"], np.float32)
        m = np.asarray(inputs["mask"], np.float32)
        mb = m > 0.5
        data = np.empty((B, 2 * T), NPBF)
        data[:, :T] = np.where(mb, x, xl).astype(NPBF)
        itm = it.astype(NPBF).view(np.uint16)
        itm = (itm & np.uint16(0xFFFE)) | mb.astype(np.uint16)
        data[:, T:].view(np.uint16)[...] = itm
        return data

    def __call__(self, inputs):
        data = self.pack_data(inputs)
        wdev = self.weights_on_device(inputs)
        args = [data if n == "data" else wdev[n]
                for n in self.in_names]
        zeros = [np.zeros((NCORES * z.shape[0], *z.shape[1:]), z.dtype)
                 for z in self.zero_outs]
        outs = self.sharded(*args, *zeros)
        return np.asarray(outs[self.out_names.index("out")]).astype(np.float32)


_runner_cache = {}


def _get_runner(t_steps=T, reps=1):
    key = (MM_MODE, ABLATE, t_steps, reps)
    if key not in _runner_cache:
        _runner_cache[key] = _Runner(t_steps, reps)
    return _runner_cache[key]


def kernel(**inputs):
    return _get_runner()(inputs)
